# revision 10
# baseline (speedup 1.0000x reference)
"""Compensated sparse linear: out = x @ (W + delta_B)^T + b on 8 NeuronCores.

Both terms of the reference contract x against [out, in] matrices, so the
whole module is one GEMM with V = W + delta_B, plus bias.

Sharding (hardcoded for x:[4,2048,4096], W/delta_B:[4096,4096], b:[4096]):
  2 token shards x 4 out-feature shards -> 8 cores; core = r*4 + c.
  Per core: x2d shard [4096, 4096], V shard [1024, 4096] -> out [4096, 1024].

Device kernel (per core): V^T shard resident in SBUF as bf16 (8.4MB),
x^T streamed in token-blocks of 128, also bf16. TensorE contracts over the
partition dim, so the host pre-tiles both operands K-major:
  xt[tb, p, kt, t] = x2d[tb*128 + t, kt*128 + p]   (8KB/partition contiguous)
  vt[p, kt, n]     = V[n, kt*128 + p]              (2KB/partition per kt slice)
bf16 matmuls stream 1 row/cycle (measured 216ns per 512-row matmul, vs 213
ideal); PSUM accumulates fp32, rel err ~2.3e-3 against the fp32 reference.
Bias is added by VectorE during the PSUM->SBUF copyback (replicated across
partitions host-side since it varies along the free dim).

Engine split so input and weight streams ride different DMA queues: SP (sync)
issues the xt stream, Activation (scalar) issues the V^T kt-slices and the
output writebacks.

PE executes in order, so a group-sequential warmup would stall on each V^T
kt-slice still in flight. Instead the first 6 groups (3 pinned t-blocks x 2
n-halves, one PSUM bank each) advance SLICE-MAJOR: every arriving kt-slice
feeds 6 matmuls (~1.3us of PE work per ~0.7us of DMA), so the PE saturates
while V^T streams in behind it.
"""

import numpy as np

import concourse.tile as tile
from concourse import bacc, mybir
from concourse.bass_utils import run_bass_kernel_spmd

try:
    import ml_dtypes

    _BF16 = ml_dtypes.bfloat16
except ImportError:  # pragma: no cover - ml_dtypes ships with jax
    _BF16 = None

P = 128
B, S, D_IN, D_OUT = 4, 2048, 4096, 4096
T = B * S
TR, NCOLS = 2, 4            # token shards x feature shards
T_C, N_C = T // TR, D_OUT // NCOLS
K = D_IN
TB = 128                    # tokens per t-block (psum partition dim)
NF = 512                    # matmul moving free dim (one PSUM bank, fp32)
KT = K // P
TBN = T_C // TB
NH = N_C // NF
PIN = 3                     # t-blocks pinned; PIN*NH chains run in the wave


def _to_bf16(a):
    a = np.asarray(a, np.float32)
    if _BF16 is not None:
        return a.astype(_BF16)
    u = a.view(np.uint32)
    rounded = (u + 0x7FFF + ((u >> 16) & 1)) >> 16
    return rounded.astype(np.uint16)


def build_nc(reps=1):
    nc = bacc.Bacc("TRN2", target_bir_lowering=False, debug=False, num_devices=8)
    xt_d = nc.dram_tensor("xt", [TBN, P, KT, TB], mybir.dt.bfloat16, kind="ExternalInput").ap()
    vt_d = nc.dram_tensor("vt", [P, KT, N_C], mybir.dt.bfloat16, kind="ExternalInput").ap()
    b_d = nc.dram_tensor("bias", [P, N_C], mybir.dt.float32, kind="ExternalInput").ap()
    out_d = nc.dram_tensor("out", [T_C, N_C], mybir.dt.float32, kind="ExternalOutput").ap()

    with tile.TileContext(nc) as tc:
        with (
            tc.tile_pool(name="vt", bufs=2) as vt_pool,
            tc.tile_pool(name="bias", bufs=1) as b_pool,
            tc.tile_pool(name="xt", bufs=PIN + 1) as xt_pool,
            tc.tile_pool(name="outp", bufs=8) as out_pool,
            tc.tile_pool(name="psum", bufs=6, space="PSUM") as psum_pool,
        ):
            bias_s = b_pool.tile([P, N_C], mybir.dt.float32)

            def drain(ps, tb, nh):
                out_s = out_pool.tile([P, NF], mybir.dt.float32)
                nc.vector.tensor_add(out_s[:], ps[:], bias_s[:, nh * NF:(nh + 1) * NF])
                nc.scalar.dma_start(
                    out_d[tb * TB:(tb + 1) * TB, nh * NF:(nh + 1) * NF], out_s[:]
                )

            def mm_group(xt_s, vt_s, tb, nh):
                ps = psum_pool.tile([P, NF], mybir.dt.float32, name="ps")
                for kt in range(KT):
                    nc.tensor.matmul(
                        ps[:], xt_s[:, kt, :], vt_s[:, kt, nh * NF:(nh + 1) * NF],
                        start=(kt == 0), stop=(kt == KT - 1),
                    )
                drain(ps, tb, nh)

            for rep in range(reps):
                # SP queue: pinned xt tiles in kt-chunks, chunk-major so
                # every pin's first chunk lands before any second chunk --
                # the wave's kt=0 step needs all PIN stationaries at once.
                KC = 8                      # kt per chunk
                pin_tiles = {}
                for tb in range(PIN):
                    pin_tiles[tb] = xt_pool.tile(
                        [P, KT, TB], mybir.dt.bfloat16, name="xt_s"
                    )
                for c in range(0, KT, KC):
                    for tb in range(PIN):
                        nc.sync.dma_start(
                            pin_tiles[tb][:, c:c + KC, :], xt_d[tb, :, c:c + KC, :]
                        )
                nc.sync.dma_start(bias_s[:], b_d[:])

                # Activation queue: V^T kt-slices, in consumption order
                vt_s = vt_pool.tile([P, KT, N_C], mybir.dt.bfloat16, name="vt")
                for kt in range(KT):
                    nc.scalar.dma_start(vt_s[:, kt, :], vt_d[:, kt, :])

                # wave warmup: PIN*NH chains advance slice-major
                wave = [(tb, nh) for tb in range(PIN) for nh in range(NH)]
                # 6 successive ring slots of the same pool name: the wave
                # occupies all 6 banks, then steady chains reuse them with
                # 5 chains of release margin.
                pss = {
                    (tb, nh): psum_pool.tile([P, NF], mybir.dt.float32, name="ps")
                    for tb, nh in wave
                }
                for kt in range(KT):
                    for tb, nh in wave:
                        nc.tensor.matmul(
                            pss[(tb, nh)][:],
                            pin_tiles[tb][:, kt, :],
                            vt_s[:, kt, nh * NF:(nh + 1) * NF],
                            start=(kt == 0), stop=(kt == KT - 1),
                        )
                for tb, nh in wave:
                    drain(pss[(tb, nh)], tb, nh)

                for tb in range(PIN, TBN):     # streamed t-blocks
                    xt_s = xt_pool.tile([P, KT, TB], mybir.dt.bfloat16)
                    nc.sync.dma_start(xt_s[:], xt_d[tb])
                    for nh in range(NH):
                        mm_group(xt_s, vt_s, tb, nh)
    nc.compile()
    return nc


def shard_layout():
    return [(r, c) for r in range(TR) for c in range(NCOLS)]


def prepare_in_maps(x, W, b, delta_B):
    x2d = np.asarray(x, np.float32).reshape(T, D_IN)
    V = np.asarray(W, np.float32) + np.asarray(delta_B, np.float32)
    b = np.asarray(b, np.float32)
    x2d_bf = _to_bf16(x2d)
    V_bf = _to_bf16(V)

    in_maps = []
    for r, c in shard_layout():
        xs = x2d_bf[r * T_C:(r + 1) * T_C]
        xt = np.ascontiguousarray(xs.reshape(TBN, TB, KT, P).transpose(0, 3, 2, 1))
        Vc = V_bf[c * N_C:(c + 1) * N_C]
        vt = np.ascontiguousarray(Vc.reshape(N_C, KT, P).transpose(2, 1, 0))
        bias = np.ascontiguousarray(np.broadcast_to(b[c * N_C:(c + 1) * N_C], (P, N_C)))
        in_maps.append({"xt": xt, "vt": vt, "bias": bias})
    return in_maps


def assemble_output(results):
    out = np.empty((T, D_OUT), np.float32)
    for i, (r, c) in enumerate(shard_layout()):
        out[r * T_C:(r + 1) * T_C, c * N_C:(c + 1) * N_C] = results[i]["out"]
    return out.reshape(B, S, D_OUT)


def kernel(x, W, b, delta_B):
    nc = build_nc()
    in_maps = prepare_in_maps(x, W, b, delta_B)
    res = run_bass_kernel_spmd(nc, in_maps, list(range(8)))
    return assemble_output(res.results)
